# revision 12
# baseline (speedup 1.0000x reference)
"""Trainium2 Bass kernel for MoEHurricaneLSTM.

Strategy: data-parallel over batch across 8 NeuronCores (256 rows each, all
weights replicated). Everything on-chip runs in "transposed" layout — the
hidden/gate dimension lives on SBUF partitions and the batch (256) is the
matmul moving free dimension — so the LSTM state h^T feeds the next step's
matmul directly and no transposes are ever needed. Matmuls use float32r
(full-rate fp32 with reduced multiply precision); the expert MLP weights are
bf16 to fit SBUF. sigmoid(x) is computed as 0.5*(tanh(x/2)+1) folded into
ACT scale/bias + scalar_tensor_tensor ops, with the cell state stored as
C = 2c and all h-consuming weights pre-scaled by 0.5 on the host, so only
one ACT table set (exp/tanh/relu) is ever loaded.
"""
import sys
import json

sys.path.insert(0, "/opt/trn_rl_repo")

import numpy as np
import ml_dtypes

import concourse.bass as bass
import concourse.tile as tile
from concourse import mybir
from concourse.bass_utils import run_bass_kernel_spmd

B, S, F, H, E, T, HH = 2048, 64, 16, 512, 8, 8, 256
NCORES = 8
BC = B // NCORES  # 256

R32 = mybir.dt.float32r
F32 = mybir.dt.float32
BF16 = mybir.dt.bfloat16
AF = mybir.ActivationFunctionType
ALU = mybir.AluOpType


# --------------------------------------------------------------------------
# BIR fixup: the walrus in this toolchain accepts at most ONE sync wait per
# instruction; Tile emits more on some instructions (notably the exit drain).
# Split extra waits onto preceding EventSemaphore instructions (same engine,
# so ordering semantics are identical).
# --------------------------------------------------------------------------
def _split_multiwait(m):
    changed = False
    for f in m.get("functions", []):
        for bb in f.get("blocks", []):
            out = []
            for inst in bb["instructions"]:
                si = inst.get("sync_info")
                waits = (si or {}).get("on_wait") or []
                if len(waits) > 1:
                    changed = True
                    extra, keep = waits[:-1], waits[-1:]
                    for j, w in enumerate(extra):
                        out.append({
                            "debug": inst.get("debug", 0),
                            "engine": inst["engine"],
                            "ins": [], "outs": [],
                            "name": f"{inst['name']}-w{j}",
                            "opcode": "EventSemaphore",
                            "sync_info": {"on_update": [], "on_wait": [w]},
                        })
                    si["on_wait"] = keep
                out.append(inst)
            bb["instructions"] = out
    return changed


def _install_birfix():
    if getattr(bass.Bass.to_json_bytes, "_multiwait_fix", False):
        return
    orig = bass.Bass.to_json_bytes

    def to_json_bytes(self, *a, **k):
        raw = orig(self, *a, **k)
        m = json.loads(raw)
        if _split_multiwait(m):
            raw = json.dumps(m).encode()
        return raw

    to_json_bytes._multiwait_fix = True
    bass.Bass.to_json_bytes = to_json_bytes


_install_birfix()


# --------------------------------------------------------------------------
# Bass program
# --------------------------------------------------------------------------
def build_nc(n_enc=S, n_dec=T):
    nc = bass.Bass("TRN2", target_bir_lowering=False, debug=False,
                   num_devices=NCORES)
    dp = nc.declare_dram_parameter

    xt_d = dp("xt", [128, n_enc // 2, 256], R32, isOutput=False)
    e0w_d = dp("e0w", [128, 4 * 2048], R32, isOutput=False)
    x0w_d = dp("x0w", [128, 2048], R32, isOutput=False)
    e1w_d = dp("e1w", [128, 8 * 2048], R32, isOutput=False)
    dw_d = dp("dw", [128, 4 * 2048], R32, isOutput=False)
    dxw_d = dp("dxw", [4, 2048], R32, isOutput=False)
    g1w_d = dp("g1w", [128, 4 * 256], R32, isOutput=False)
    g2w_d = dp("g2w", [128, 2 * 8], R32, isOutput=False)
    w1t_d = dp("w1t", [128, 4 * 6144], BF16, isOutput=False)
    w2t_d = dp("w2t", [128, 48 * 4], BF16, isOutput=False)
    b0_d = dp("b0", [128, 16], F32, isOutput=False)
    b1_d = dp("b1", [128, 16], F32, isOutput=False)
    bd_d = dp("bd", [128, 16], F32, isOutput=False)
    gb1_d = dp("gb1", [128, 2], F32, isOutput=False)
    gb2_d = dp("gb2", [8, 1], F32, isOutput=False)
    b1f_d = dp("b1f", [128, 48], F32, isOutput=False)
    b2a_d = dp("b2a", [8, 5], R32, isOutput=False)
    z0_d = dp("z0", [128, 4, 256], R32, isOutput=False)

    do_d = dp("dec_out", [4, n_dec, 256], R32, isOutput=True)
    ga_d = dp("gates", [8, 256], F32, isOutput=True)

    escr = nc.dram_tensor("escr", [8, 256], R32)
    rscr = nc.dram_tensor("rscr", [1, 256], F32)

    with tile.TileContext(nc) as tc:
        from contextlib import ExitStack
        with ExitStack() as octx:
            # ---------------- persistent pools -------------------------
            pers = octx.enter_context(tc.tile_pool(name="pers", bufs=1))
            stag = octx.enter_context(tc.tile_pool(name="stag", bufs=2))
            tmp = octx.enter_context(tc.tile_pool(name="tmp", bufs=1))
            psum = octx.enter_context(
                tc.tile_pool(name="psum", bufs=8, space="PSUM"))

            h0 = pers.tile([128, 4, 256], R32)
            c0 = pers.tile([128, 4, 256], F32)
            h1 = pers.tile([128, 4, 256], R32)
            c1 = pers.tile([128, 4, 256], F32)
            outs = pers.tile([4, n_dec + 1, 256], R32)
            gacc = pers.tile([8, 256], F32)

            nc.sync.dma_start(out=h0[:], in_=z0_d[:])
            nc.sync.dma_start(out=h1[:], in_=z0_d[:])
            nc.vector.memset(c0[:], 0.0)
            nc.vector.memset(c1[:], 0.0)
            nc.vector.memset(gacc[:], 0.0)
            nc.sync.dma_start(out=outs[:, 0, :], in_=z0_d[0:4, 0, :])

            def lstm_ew(tg_i, tg_f, tg_g, tg_o, C, Hst):
                """Gate chain on [128, 4*256] views. C = 2c (fp32),
                Hst = 2h (f32r)."""
                u = tmp.tile([128, 4, 256], F32, tag="u")
                v = tmp.tile([128, 4, 256], F32, tag="v")
                tcl = tmp.tile([128, 4, 256], F32, tag="tc")
                # u = (tanh_f + 1) * C
                nc.vector.scalar_tensor_tensor(
                    out=u[:], in0=tg_f[:], scalar=1.0, in1=C[:],
                    op0=ALU.add, op1=ALU.mult)
                # v = (tanh_i + 1) * tanh_g
                nc.vector.scalar_tensor_tensor(
                    out=v[:], in0=tg_i[:], scalar=1.0, in1=tg_g[:],
                    op0=ALU.add, op1=ALU.mult)
                # C' = 0.5*u + v
                nc.vector.scalar_tensor_tensor(
                    out=C[:], in0=u[:], scalar=0.5, in1=v[:],
                    op0=ALU.mult, op1=ALU.add)
                # tanh(c') = tanh(0.5 * C')
                nc.scalar.activation(tcl[:], C[:], AF.Tanh, bias=0.0, scale=0.5)
                # H = (tanh_o + 1) * tanh(c')   (= 2h; weights pre-scaled)
                nc.vector.scalar_tensor_tensor(
                    out=Hst[:], in0=tg_o[:], scalar=1.0, in1=tcl[:],
                    op0=ALU.add, op1=ALU.mult)

            def lstm_step(wt, nkt, rhs_of_k, xk_pair, bias, Hst, C):
                """One LSTM layer step. wt: [128, nkt, 2048] f32r weights;
                rhs_of_k(k) -> moving [*, 256]; xk_pair = (stationary,
                moving) extra input k-tile or None."""
                tgs = [stag.tile([128, 4, 256], F32, tag=f"tg{i}", name=f"tg{i}")
                       for i in range(4)]
                for g in range(4):
                    for j in range(4):
                        m = g * 4 + j
                        ps = psum.tile([128, 256], F32, bufs=5)
                        first = True
                        if xk_pair is not None:
                            st, mv = xk_pair
                            nc.tensor.matmul(ps[:], st[:, m * 128:(m + 1) * 128],
                                             mv, start=True, stop=False,
                                             skip_group_check=True)
                            first = False
                        for k in range(nkt):
                            nc.tensor.matmul(
                                ps[:], wt[:, k, m * 128:(m + 1) * 128],
                                rhs_of_k(k), start=first,
                                stop=(k == nkt - 1), skip_group_check=True)
                            first = False
                        sc = 1.0 if g == 2 else 0.5
                        nc.scalar.activation(tgs[g][:, j, :], ps[:], AF.Tanh,
                                             bias=bias[:, m:m + 1], scale=sc)
                lstm_ew(tgs[0], tgs[1], tgs[2], tgs[3], C, Hst)

            # ---------------- encoder phase ---------------------------
            with ExitStack() as ectx:
                encw = ectx.enter_context(tc.tile_pool(name="encw", bufs=1))
                e0w = encw.tile([128, 4, 2048], R32)
                x0w = encw.tile([128, 2048], R32)
                e1w = encw.tile([128, 8, 2048], R32)
                xts = encw.tile([128, n_enc // 2, 256], R32)
                b0 = encw.tile([128, 16], F32)
                b1 = encw.tile([128, 16], F32)
                nc.sync.dma_start(out=e0w[:], in_=e0w_d[:].rearrange(
                    "p (a b) -> p a b", a=4))
                nc.sync.dma_start(out=x0w[:], in_=x0w_d[:])
                nc.sync.dma_start(out=e1w[:], in_=e1w_d[:].rearrange(
                    "p (a b) -> p a b", a=8))
                nc.sync.dma_start(out=xts[:], in_=xt_d[:])
                nc.sync.dma_start(out=b0[:], in_=b0_d[:])
                nc.sync.dma_start(out=b1[:], in_=b1_d[:])

                for s in range(n_enc):
                    ph = (s % 2) * 64
                    xk = xts[ph:ph + 64, s // 2, :]
                    xst = x0w[ph:ph + 64, :]
                    lstm_step(e0w, 4, lambda k: h0[:, k, :], (xst, xk),
                              b0, h0, c0)
                    # layer 1: k 0-3 input = h0 (this step), 4-7 = h1.
                    # h1 tiles first — they are ready before h0 of this step.
                    order = [4, 5, 6, 7, 0, 1, 2, 3]
                    tgs = [stag.tile([128, 4, 256], F32, tag=f"tg{i}", name=f"tg{i}")
                           for i in range(4)]
                    for g in range(4):
                        for j in range(4):
                            m = g * 4 + j
                            ps = psum.tile([128, 256], F32, bufs=5)
                            for i, k in enumerate(order):
                                rhs = h1[:, k - 4, :] if k >= 4 else h0[:, k, :]
                                nc.tensor.matmul(
                                    ps[:], e1w[:, k, m * 128:(m + 1) * 128],
                                    rhs, start=(i == 0), stop=(i == 7),
                                    skip_group_check=True)
                            sc = 1.0 if g == 2 else 0.5
                            nc.scalar.activation(tgs[g][:, j, :], ps[:],
                                                 AF.Tanh, bias=b1[:, m:m + 1],
                                                 scale=sc)
                    lstm_ew(tgs[0], tgs[1], tgs[2], tgs[3], c1, h1)

            # ---------------- decoder phase ---------------------------
            with ExitStack() as dctx:
                decw = dctx.enter_context(tc.tile_pool(name="decw", bufs=1))
                dpool = dctx.enter_context(tc.tile_pool(name="dpool", bufs=2))
                dper = dctx.enter_context(tc.tile_pool(name="dper", bufs=1))
                dw = decw.tile([128, 4, 2048], R32)
                dxw = decw.tile([4, 2048], R32)
                g1w = decw.tile([128, 4, 256], R32)
                g2w = decw.tile([128, 2, 8], R32)
                w1t = decw.tile([128, 4, 6144], BF16)
                w2t = decw.tile([128, 48, 4], BF16)
                bd = decw.tile([128, 16], F32)
                gb1 = decw.tile([128, 2], F32)
                gb2 = decw.tile([8, 1], F32)
                b1f = decw.tile([128, 48], F32)
                b2a = decw.tile([8, 5], R32)
                nc.sync.dma_start(out=dw[:], in_=dw_d[:].rearrange(
                    "p (a b) -> p a b", a=4))
                nc.sync.dma_start(out=dxw[:], in_=dxw_d[:])
                nc.sync.dma_start(out=g1w[:], in_=g1w_d[:].rearrange(
                    "p (a b) -> p a b", a=4))
                nc.sync.dma_start(out=g2w[:], in_=g2w_d[:].rearrange(
                    "p (a b) -> p a b", a=2))
                nc.sync.dma_start(out=w1t[:], in_=w1t_d[:].rearrange(
                    "p (a b) -> p a b", a=4))
                nc.sync.dma_start(out=w2t[:], in_=w2t_d[:].rearrange(
                    "p (a b) -> p a b", a=48))
                nc.sync.dma_start(out=bd[:], in_=bd_d[:])
                nc.sync.dma_start(out=gb1[:], in_=gb1_d[:])
                nc.sync.dma_start(out=gb2[:], in_=gb2_d[:])
                nc.sync.dma_start(out=b1f[:], in_=b1f_d[:])
                nc.sync.dma_start(out=b2a[:], in_=b2a_d[:])

                for t in range(n_dec):
                    din = outs[:, t, :]
                    lstm_step(dw, 4, lambda k: h1[:, k, :], (dxw, din),
                              bd, h1, c1)
                    # ctx = h1. Cast to bf16 for the expert matmuls.
                    ctxb = dpool.tile([128, 4, 256], BF16, tag="ctxb")
                    nc.vector.tensor_copy(ctxb[:], h1[:].bitcast(F32))
                    # gating MLP: r1 = relu(gW1 ctx + gb1)
                    r1 = dpool.tile([128, 2, 256], R32, tag="r1")
                    for m in range(2):
                        ps = psum.tile([128, 256], F32, bufs=5)
                        for k in range(4):
                            nc.tensor.matmul(ps[:],
                                             g1w[:, k, m * 128:(m + 1) * 128],
                                             h1[:, k, :], start=(k == 0),
                                             stop=(k == 3),
                                             skip_group_check=True)
                        nc.scalar.activation(r1[:, m, :], ps[:], AF.Relu,
                                             bias=gb1[:, m:m + 1], scale=1.0)
                    # logits -> unnormalized exp weights eh [8, 256]
                    ps8 = psum.tile([8, 256], F32, bufs=1)
                    for k in range(2):
                        nc.tensor.matmul(ps8[:], g2w[:, k, :], r1[:, k, :],
                                         start=(k == 0), stop=(k == 1),
                                         skip_group_check=True)
                    eh = dpool.tile([8, 256], R32, tag="eh")
                    nc.scalar.activation(eh[:], ps8[:], AF.Exp,
                                         bias=gb2[:, 0:1], scale=1.0)
                    # bounce eh through DRAM to build the per-expert
                    # partition-broadcast Gb [128, 8, 2, 256]
                    nc.sync.dma_start(out=escr[:], in_=eh[:])
                    gbes = []
                    for e in range(8):
                        gbe = dpool.tile([128, 256], R32, tag="gbe",
                                         name=f"gbe{e}", bufs=8)
                        esrc = bass.AP(tensor=escr, offset=e * 256,
                                       ap=[[0, 128], [1, 256]])
                        nc.sync.dma_start(out=gbe[:], in_=esrc)
                        gbes.append(gbe)

                    # expert stack + weighted combine
                    ps4 = psum.tile([4, 256], F32, bufs=1)
                    pse = psum.tile([1, 256], F32, bufs=1)
                    nc.tensor.matmul(pse[:], b2a[:, 4:5], eh[:], start=True,
                                     stop=True, skip_group_check=True)
                    nc.tensor.matmul(ps4[:], b2a[:, 0:4], eh[:], start=True,
                                     stop=False, skip_group_check=True)
                    for m in range(48):
                        ps = psum.tile([128, 256], F32, bufs=5)
                        for k in range(4):
                            nc.tensor.matmul(ps[:],
                                             w1t[:, k, m * 128:(m + 1) * 128],
                                             ctxb[:, k, :], start=(k == 0),
                                             stop=(k == 3),
                                             skip_group_check=True)
                        rh = dpool.tile([128, 256], F32, tag="rh", bufs=6)
                        nc.scalar.activation(rh[:], ps[:], AF.Relu,
                                             bias=b1f[:, m:m + 1], scale=1.0)
                        gh = dpool.tile([128, 256], BF16, tag="gh", bufs=6)
                        e = (m % 16) // 2
                        nc.vector.tensor_tensor(
                            out=gh[:], in0=rh[:],
                            in1=gbes[e][:].bitcast(F32),
                            op=ALU.mult)
                        nc.tensor.matmul(ps4[:], w2t[:, m, :], gh[:],
                                         start=False, stop=(m == 47),
                                         skip_group_check=True)
                    # normalize by sumexp (row 4 of ps5) via DRAM bounce
                    rec = dpool.tile([1, 256], F32, tag="rec")
                    nc.vector.reciprocal(out=rec[:], in_=pse[:])
                    nc.sync.dma_start(out=rscr[:], in_=rec[:])
                    rb8 = dpool.tile([8, 256], F32, tag="rb8")
                    rsrc = bass.AP(tensor=rscr, offset=0,
                                   ap=[[0, 8], [1, 256]])
                    nc.sync.dma_start(out=rb8[:], in_=rsrc)
                    # dec_in(t+1) = out4 / sumexp  (also the step outputs)
                    nc.vector.tensor_tensor(out=outs[:, t + 1, :],
                                            in0=ps4[:], in1=rb8[0:4, :],
                                            op=ALU.mult)
                    # gates_avg accumulation: gacc += eh / sumexp
                    gn = dpool.tile([8, 256], F32, tag="gn")
                    nc.vector.tensor_tensor(out=gn[:], in0=eh[:].bitcast(F32),
                                            in1=rb8[:], op=ALU.mult)
                    nc.vector.tensor_tensor(out=gacc[:], in0=gacc[:],
                                            in1=gn[:], op=ALU.add)

                nc.sync.dma_start(out=do_d[:], in_=outs[:, 1:, :])
                nc.sync.dma_start(out=ga_d[:], in_=gacc[:])

    return nc


# --------------------------------------------------------------------------
# Host-side packing
# --------------------------------------------------------------------------
def _pack_weights(inp):
    f32 = np.float32

    def wT(w, scale):
        # [out, in] -> [in, out] scaled, tiled to [128, in//128, out]
        wt = (w.T * scale).astype(f32)
        kin = wt.shape[0]
        return np.ascontiguousarray(
            wt.reshape(kin // 128, 128, wt.shape[1]).transpose(1, 0, 2))

    def bias_s(b):
        # [2048] -> [128, 16] cols = m-tiles; i,f,o gates scaled by 0.5
        bs = b.astype(f32).copy()
        sc = np.ones(16, f32) * 0.5
        sc[8:12] = 1.0  # gate g = m-tiles 8..11
        return np.ascontiguousarray(bs.reshape(16, 128).T * sc[None, :])

    out = {}
    out["e0w"] = wT(inp["enc_Whh0"], 0.5).reshape(128, -1)
    x0w = np.zeros((64, 2048), f32)
    x0w[:16] = (inp["enc_Wih0"].T * 1.0).astype(f32)
    out["x0w"] = np.tile(x0w, (2, 1))
    e1 = np.concatenate([wT(inp["enc_Wih1"], 0.5), wT(inp["enc_Whh1"], 0.5)],
                        axis=1)
    out["e1w"] = np.ascontiguousarray(e1).reshape(128, -1)
    out["dw"] = wT(inp["dec_Whh"], 0.5).reshape(128, -1)
    out["dxw"] = np.ascontiguousarray(inp["dec_Wih"].T.astype(f32))
    out["g1w"] = wT(inp["gW1"], 0.5).reshape(128, -1)
    out["g2w"] = wT(inp["gW2"], 1.0).reshape(128, -1)

    W1f = np.concatenate([inp["tW1"].reshape(E * HH, H),
                          inp["iW1"].reshape(E * HH, H),
                          inp["wW1"].reshape(E * HH, H)], 0)
    out["w1t"] = wT(W1f, 0.5).reshape(128, -1).astype(ml_dtypes.bfloat16)
    W2cat = np.zeros((3 * E * HH, 4), f32)
    for e in range(E):
        W2cat[e * HH:(e + 1) * HH, 0:2] = inp["tW2"][e].T
        W2cat[E * HH + e * HH:E * HH + (e + 1) * HH, 2:3] = inp["iW2"][e].T
        W2cat[2 * E * HH + e * HH:2 * E * HH + (e + 1) * HH, 3:4] = \
            inp["wW2"][e].T
    out["w2t"] = np.ascontiguousarray(
        W2cat.reshape(48, 128, 4).transpose(1, 0, 2)
    ).reshape(128, -1).astype(ml_dtypes.bfloat16)

    out["b0"] = bias_s(inp["enc_b0"])
    out["b1"] = bias_s(inp["enc_b1"])
    out["bd"] = bias_s(inp["dec_b"])
    out["gb1"] = np.ascontiguousarray(
        inp["gb1"].astype(f32).reshape(2, 128).T)
    out["gb2"] = inp["gb2"].astype(f32).reshape(8, 1)
    b1fl = np.concatenate([inp["tb1"].reshape(-1), inp["ib1"].reshape(-1),
                           inp["wb1"].reshape(-1)]).astype(f32)
    out["b1f"] = np.ascontiguousarray(b1fl.reshape(48, 128).T)
    b2a = np.ones((8, 5), f32)
    b2a[:, 0:2] = inp["tb2"].astype(f32)
    b2a[:, 2:3] = inp["ib2"].astype(f32)
    b2a[:, 3:4] = inp["wb2"].astype(f32)
    out["b2a"] = b2a
    out["z0"] = np.zeros((128, 4, 256), f32)
    return out


def _pack_x(x_shard, n_enc=S):
    # [BC, n_enc, F] -> [128, n_enc//2, 256]: partition (s%2)*64+f, free
    # (s//2, b); rows 16..63 of each 64-block are zero padding.
    f32 = np.float32
    xt = np.zeros((128, n_enc // 2, BC), f32)
    xr = x_shard.astype(f32).transpose(2, 1, 0)  # [F, S, B]
    for ph in range(2):
        xt[ph * 64:ph * 64 + 16] = xr[:, ph::2, :]
    return xt


_CACHE = {}


def kernel(**inputs):
    if "nc" not in _CACHE:
        _CACHE["nc"] = build_nc()
    nc = _CACHE["nc"]
    w = _pack_weights(inputs)
    x = np.asarray(inputs["x"], np.float32)
    in_maps = []
    for c in range(NCORES):
        m = dict(w)
        m["xt"] = _pack_x(x[c * BC:(c + 1) * BC])
        in_maps.append(m)
    res = run_bass_kernel_spmd(nc, in_maps, list(range(NCORES)))

    tracks, intens, winds, gavgs = [], [], [], []
    for c in range(NCORES):
        do = res.results[c]["dec_out"]          # [4, T, 256]
        ga = res.results[c]["gates"] / T        # [8, 256]
        dlat = do[0].T                          # [256, T]
        dlon = do[1].T
        tracks.append(np.concatenate([dlat, dlon], axis=1))
        intens.append(do[2].T)
        winds.append(do[3].T)
        gavgs.append(ga.T)
    track = np.concatenate(tracks, 0).astype(np.float32)
    intensity = np.concatenate(intens, 0).astype(np.float32)
    wind_abs = np.concatenate(winds, 0).astype(np.float32)
    gates_avg = np.concatenate(gavgs, 0).astype(np.float32)
    return track, intensity, wind_abs, gates_avg
